# revision 30
# baseline (speedup 1.0000x reference)
"""Trainium2 Bass kernel for batched masked attention (Z=8, S=2048, D=1024).

Strategy: pure data-parallel over batch z — each of the 8 NeuronCores computes
full attention for one batch element. No collectives.

Mask compaction: the reference's symmetric mask kills row q and column k
whenever position is masked (masked-query rows are exactly 0 in the output,
masked-key columns contribute exactly 0 to every sum). Query-mask == key-mask,
so the host gathers only the unmasked positions (~half), padded to a multiple
of 32 shared across cores, runs dense attention on the compacted sequence,
and scatters the result rows back into a zero output. Bit-equivalent math at
~40% of the dense FLOPs.

Score-projection fusion: z = (x Wq^T)(x Wk^T)^T = x (Wq^T Wk) x^T, so the
host precomputes G = Wq^T @ Wk once (f32) and the kernel runs a single score
projection m = x @ G instead of separate q and k projections — the z matmul
then contracts the already-resident xcT tiles against mT. One fewer GEMM on
the PE and 2MB less input DMA.

Per-core dataflow (all matmuls, no on-chip transposes):
  - host passes xcT = x[z][idx].T  [D, N] (bf16), G = Wq^T Wk (bf16),
    Wv.T (bf16), bv (f32)
  - mT[j,s]           = G-tile.T @ xcT         (PE)
  - v[s,a]            = xcT-tile.T @ Wv.T + bv (bias added on DVE from a
                                                partition-broadcast bv row)
  - zT[k,q]           = xcT-tile.T @ mT        (scores with keys on partitions)
  - ET                = exp(zT/32 + kbias[k])  (ScalarE; padding keys get
                                                bias -30000 -> exp underflows to 0)
  - out_psum[q,a]     = ET-tile.T @ v          (PE, contraction over keys)
  - denom[q]          = ET-tile.T @ ones       (same stationary, N=1 matmul)
  - out[q,a]          = out_psum / denom[q]    (chunk 0 on DVE, chunk 1 on
                                                ScalarE Copy+scale; bf16 out)

Scheduling notes (why this is fast):
  - The first m-projection pass runs it-OUTER (contraction index outer, 8
    open PSUM chains inner) so chain step `it` depends only on x-tile it and
    wm-tile it; the first x/wm tiles are additionally split into 512-column
    pieces (separate tiles -> exact dependencies), so the first real matmul
    needs just 256KB of DMA.
  - Input DMA descriptor generation (~620ns per dma_start per sequencer) is
    split across the two HWDGE engines: sync issues the x tiles, scalar
    issues wm then wv, both in first-consumed order.
  - All later projection passes run one chain at a time so the PSUM->SBUF
    drains (DVE copies / bias-adds) trickle instead of bursting against the
    8-bank PSUM ring.
  - PE warmup matmuls cover the DMA lead-in so the PE p-state ramp (0.65 ->
    1.2 -> 2.4 GHz over ~3us of continuous execution) is never reset by an
    idle gap; real matmuls start the moment data lands.
  - Output is stored as bf16 (halves store bytes); the final q-subtile is
    the small straggler so the post-PE tail is short.
"""

import numpy as np
import ml_dtypes

P = 128
S = 2048  # full sequence length
D = 1024  # model dim (= dim_qk = dim_v)
NI = D // P  # 8 contraction tiles for projections
VC = 512  # v free-dim chunk
NVC = D // VC  # 2
SCALE = 1.0 / 32.0  # 1/sqrt(D)
GRAN = 32  # sequence padding granularity
# PE pre-warm dummy matmuls bridge until the first input tiles land. The
# count must cover the WORST-CASE DMA arrival: if warmup ends before the
# data and the PE idles even ~2us, the power-management throttle drops the
# clock for the remainder of the run (~+30us observed) — overshooting by a
# few matmuls costs only ~50ns each. With the packed 3KB-row first tile the
# worst-case arrival is ~11us.
NWARM = 44

_CACHE = {}


def _chunks(total, maxw):
    out = []
    off = 0
    while off < total:
        w = min(maxw, total - off)
        out.append((off, w))
        off += w
    return out


def _build_nc(N):
    """Build the per-core graph for a compacted, padded sequence length N."""
    from contextlib import ExitStack

    import concourse.tile as tile
    from concourse import bacc, mybir
    from concourse.bass import ts, ds

    f32 = mybir.dt.float32
    bf16 = mybir.dt.bfloat16
    EXP = mybir.ActivationFunctionType.Exp
    COPY = mybir.ActivationFunctionType.Copy

    ktiles = _chunks(N, P)  # [(koff, kh)]
    nkt = len(ktiles)
    qchunks = _chunks(N, 512)

    nc = bacc.Bacc(None, target_bir_lowering=False, debug=False)

    WA = min(512, N)  # x a-piece width (first m-proj chunk)
    WB = N - WA
    # xwp packs [x-tile a-piece | wm tile] per it-row-block: 3KB rows halve
    # the descriptor count on the startup-critical path (early DMA is
    # latency-bound per descriptor, not per byte).
    xwp_d = nc.declare_dram_parameter("xwp", [D, WA + D], bf16, isOutput=False)
    xc_d = nc.declare_dram_parameter("xc", [D, N], bf16, isOutput=False)
    wv_d = nc.declare_dram_parameter("wv", [D, D], bf16, isOutput=False)
    bv_d = nc.declare_dram_parameter("bv", [1, D], f32, isOutput=False)
    kb_d = nc.declare_dram_parameter("kbias", [P, nkt], f32, isOutput=False)
    out_d = nc.declare_dram_parameter("out", [N, D], bf16, isOutput=True)

    with tile.TileContext(nc) as tc, ExitStack() as st:
        const = st.enter_context(tc.tile_pool(name="const", bufs=1))
        persist = st.enter_context(tc.tile_pool(name="persist", bufs=1))
        # one PSUM ring shared by every stage — no pool-boundary barriers
        ps = st.enter_context(tc.tile_pool(name="ps", bufs=8, space="PSUM"))

        def psum(name, h, w):
            t = ps.tile([P, 512], f32, name=name, tag="ps")
            return t[:h, :w]

        ones_col = const.tile([P, 1], bf16, name="ones_col", tag="ones_col")
        nc.gpsimd.memset(ones_col, 1.0)

        # PE pre-warm: dummy matmuls with no data deps run during the input
        # DMA lead-in so HAM un-throttles before the first real matmul.
        ws = const.tile([P, P], bf16, name="ws", tag="ws")
        nc.gpsimd.memset(ws, 0.0)
        for i in range(NWARM):
            wp = psum(f"wp{i}", P, P)
            nc.tensor.matmul(wp, lhsT=ws, rhs=ws, start=True, stop=True)

        # If the last k-tile is short, its ET / v tiles are padded to 128
        # rows with zeros (set once; exp/v-proj only ever write [:kh]) so
        # every PV chain step runs with full 128-partition contraction.
        kh_last = ktiles[-1][1]
        pad_straggler = kh_last < P
        et_s = []
        if pad_straggler:
            for i in range(2):
                t = persist.tile([P, 512], bf16, name=f"et_s{i}", tag="et_s", bufs=2)
                nc.gpsimd.memset(t[kh_last:, :], 0.0)
                et_s.append(t)

        # xc stays resident through phase 2 (the z matmul contracts it).
        # Tile 0 of xc and wm is split into 512-column pieces so the very
        # first accumulation steps depend on minimal DMA. All column ranges
        # used anywhere (q-chunks, k-tiles, a-blocks) stay within one piece.
        mt = [
            persist.tile([P, N], bf16, name=f"mt{a}", tag="mt", bufs=NI)
            for a in range(NI)
        ]
        v = [
            persist.tile([P, D], bf16, name=f"v{s}", tag="v", bufs=nkt)
            for s in range(nkt)
        ]
        if pad_straggler:
            nc.gpsimd.memset(v[nkt - 1][kh_last:, :], 0.0)

        # Each x tile's a-piece (cols 0:WA — first m-proj chunk, k-tiles 0-3)
        # is packed with its wm tile into one combined tile loaded by a single
        # dma_start, alternating between the two HWDGE engines in consumption
        # order. The b-pieces (cols WA:N — m-proj chunks 1+, k-tiles 4+) are
        # issued afterwards; they are not consumed until the first pass is
        # long done. This keeps the supply-critical window at 3MB in 1024
        # large rows, and the first pass never outruns the DMA.
        xwt = [None] * NI  # combined [P, WA (x) + D (wm)] tiles
        xb = [None] * NI

        def xsl(it, off, w):
            if off + w <= WA:
                return xwt[it][:, ds(off, w)]
            assert off >= WA, (off, w)
            return xb[it][:, ds(off - WA, w)]

        def wsl(it, a):
            return xwt[it][:, ds(WA + a * P, P)]

        # ---- phase 1: projections -------------------------------------
        with tc.tile_pool(name="xw", bufs=1) as xw:
            # The scalar-issued DMA ring starts ~2.4us slower than the sync
            # ring, so the first-consumed pairs (tight deadlines) go on sync
            # even though that serializes their descriptor generation; pairs
            # 4-7 have ~7us+ of deadline slack and absorb the scalar-ring lag.
            for it in range(NI):
                t = persist.tile(
                    [P, WA + D], bf16, name=f"xwt{it}", tag="xt", bufs=2 * NI
                )
                eng = nc.sync if it < 2 else nc.scalar
                eng.dma_start(t, xwp_d[ts(it, P), :])
                xwt[it] = t
            for it in range(NI):
                if WB:
                    t = persist.tile(
                        [P, WB], bf16, name=f"xb{it}", tag="xt", bufs=2 * NI
                    )
                    eng = nc.sync if it < 7 else nc.scalar
                    eng.dma_start(t, xc_d[ts(it, P), ds(WA, WB)])
                    xb[it] = t

            kb_sb = const.tile([P, nkt], f32, name="kb_sb", tag="kb_sb")
            nc.scalar.dma_start(kb_sb, kb_d[:, :])
            bv_sb = const.tile([1, D], f32, name="bv_sb", tag="bv_sb")
            nc.scalar.dma_start(bv_sb, bv_d[:, :])
            bv_bc = const.tile([P, D], f32, name="bv_bc", tag="bv_bc")
            nc.gpsimd.partition_broadcast(bv_bc, bv_sb[:1, :])

            # mT: out[a-tile, chunk] = sum_it G[it, a-tile].T @ xcT[it, chunk]
            # First chunk runs it-OUTER (8 open chains) so chain step `it`
            # needs only x/wm tile it — matched to DMA arrival order. Later
            # chunks run chain-at-a-time so the DVE copies trickle.
            off0, w0 = qchunks[0]
            pss = [psum(f"pp_m0_{a}", P, w0) for a in range(NI)]
            for it in range(NI):
                for a in range(NI):
                    nc.tensor.matmul(
                        pss[a],
                        lhsT=wsl(it, a),
                        rhs=xsl(it, off0, w0),
                        start=(it == 0),
                        stop=(it == NI - 1),
                    )
            for a in range(NI):
                nc.vector.tensor_copy(mt[a][:, ds(off0, w0)], pss[a])

            for ci in range(1, len(qchunks)):
                off, w = qchunks[ci]
                for a in range(NI):
                    pp = psum(f"pp_m{ci}_{a}", P, w)
                    for it in range(NI):
                        nc.tensor.matmul(
                            pp,
                            lhsT=wsl(it, a),
                            rhs=xsl(it, off, w),
                            start=(it == 0),
                            stop=(it == NI - 1),
                        )
                    nc.vector.tensor_copy(mt[a][:, ds(off, w)], pp)

            # wv is not consumed until the v-projection (~mid-kernel), so its
            # DMA is deferred behind a dummy dependency on the last x tile —
            # otherwise its descriptors would steal HBM bandwidth from the
            # xc/wm stream that the first m-projection pass is racing.
            wv_t = []
            wv_gate = xb[NI - 1] if WB else xa[NI - 1]
            for it in range(NI):
                w = xw.tile([P, D], bf16, name=f"wvt{it}", tag="w", bufs=18)
                nc.vector.tensor_copy(w[:1, :1], wv_gate[:1, :1])
                nc.scalar.dma_start(w, wv_d[ts(it, P), :])
                wv_t.append(w)

            # v: out[k-tile, chunk] = sum_it xcT[it, k-tile].T @ Wv.T[it, chunk] + bv
            for s16, (koff, kh) in enumerate(ktiles):
                for c in range(NVC):
                    pp = psum(f"pp_v{s16}_{c}", kh, VC)
                    for it in range(NI):
                        nc.tensor.matmul(
                            pp,
                            lhsT=xsl(it, koff, kh),
                            rhs=wv_t[it][:, ts(c, VC)],
                            start=(it == 0),
                            stop=(it == NI - 1),
                        )
                    nc.vector.tensor_add(
                        v[s16][:kh, ts(c, VC)], pp, bv_bc[:kh, ts(c, VC)]
                    )

        # ---- phase 2: attention ---------------------------------------
        with (
            tc.tile_pool(name="etp", bufs=1) as etp,
            tc.tile_pool(name="outp", bufs=4) as outp,
            tc.tile_pool(name="smol", bufs=8) as smol,
        ):
            for qc, (qoff, qw) in enumerate(qchunks):
                ets = []
                for k16, (koff, kh) in enumerate(ktiles):
                    zps = psum(f"z{qc}_{k16}", kh, qw)
                    for a in range(NI):
                        nc.tensor.matmul(
                            zps,
                            lhsT=xsl(a, koff, kh),
                            rhs=mt[a][:, ds(qoff, qw)],
                            start=(a == 0),
                            stop=(a == NI - 1),
                        )
                    if pad_straggler and k16 == nkt - 1:
                        et_full = et_s[qc % 2]
                        nc.scalar.activation(
                            et_full[:kh, :qw],
                            zps,
                            EXP,
                            bias=kb_sb[:kh, k16 : k16 + 1],
                            scale=SCALE,
                        )
                        ets.append(et_full[:, :qw])
                    else:
                        et = etp.tile(
                            [P, 512],
                            bf16,
                            name=f"et{qc}_{k16}",
                            tag="et",
                            bufs=2 * nkt,
                        )[:kh, :qw]
                        nc.scalar.activation(
                            et, zps, EXP, bias=kb_sb[:kh, k16 : k16 + 1], scale=SCALE
                        )
                        ets.append(et)

                last_sub = qc == len(qchunks) - 1
                for qsoff, qh in _chunks(qw, P):
                    qrow = qoff + qsoff  # global compacted row
                    opss = [psum(f"pv{qrow}_{c}", qh, VC) for c in range(NVC)]
                    dps = psum(f"dn{qrow}", qh, 1)
                    final = last_sub and qsoff + qh == qw
                    for k16, (koff, kh) in enumerate(ktiles):
                        khe = P if (pad_straggler and k16 == nkt - 1) else kh
                        lhs = ets[k16][:khe, ds(qsoff, qh)]
                        # order (c0, denom, c1): the subtile's last PE op is a
                        # wide matmul, which hides the next subtile's first
                        # LDWEIGHTS instead of exposing it after the 1-col
                        # denominator matmul. The kernel's very last subtile
                        # instead runs (denom, c0, c1) so the reciprocal
                        # overlaps the final two matmuls.
                        if final:
                            nc.tensor.matmul(
                                dps,
                                lhsT=lhs,
                                rhs=ones_col[:khe, :1],
                                start=(k16 == 0),
                                stop=(k16 == nkt - 1),
                            )
                        nc.tensor.matmul(
                            opss[0],
                            lhsT=lhs,
                            rhs=v[k16][:khe, ts(0, VC)],
                            start=(k16 == 0),
                            stop=(k16 == nkt - 1),
                        )
                        if not final:
                            nc.tensor.matmul(
                                dps,
                                lhsT=lhs,
                                rhs=ones_col[:khe, :1],
                                start=(k16 == 0),
                                stop=(k16 == nkt - 1),
                            )
                        nc.tensor.matmul(
                            opss[1],
                            lhsT=lhs,
                            rhs=v[k16][:khe, ts(1, VC)],
                            start=(k16 == 0),
                            stop=(k16 == nkt - 1),
                        )
                    rec = smol.tile([P, 1], f32, name=f"rec{qrow}", tag="rec")[:qh]
                    nc.vector.reciprocal(rec, dps)
                    # chunk 0 normalizes on DVE, chunk 1 on ScalarE — the two
                    # run in parallel and each engine then issues its own
                    # store, so the per-subtile epilogue is ~halved.
                    ot0 = outp.tile([P, VC], bf16, name=f"ot{qrow}_0", tag="ot")[:qh]
                    nc.vector.tensor_scalar_mul(ot0, opss[0], rec)
                    nc.sync.dma_start(out_d[ds(qrow, qh), ts(0, VC)], ot0)
                    ot1 = outp.tile([P, VC], bf16, name=f"ot{qrow}_1", tag="ot")[:qh]
                    nc.scalar.activation(ot1, opss[1], COPY, scale=rec)
                    nc.scalar.dma_start(out_d[ds(qrow, qh), ts(1, VC)], ot1)

    nc.compile()
    return nc


def _get_nc(N):
    if N not in _CACHE:
        _CACHE[N] = _build_nc(N)
    return _CACHE[N]


def _make_in_maps(x, Wq, Wk, Wv, bv, mask, idxs, N):
    bf16 = ml_dtypes.bfloat16
    ktiles = _chunks(N, P)
    nkt = len(ktiles)
    G = np.float32(Wq).T @ np.float32(Wk)  # z = x G x^T
    wm = np.ascontiguousarray(G).astype(bf16)
    wv_t = np.ascontiguousarray(Wv.astype(np.float32).T).astype(bf16)
    bv_row = np.ascontiguousarray(bv.astype(np.float32).reshape(1, D))
    in_maps = []
    for z in range(8):
        idx = idxs[z]
        n = idx.size
        idx_pad = np.zeros(N, dtype=np.int64)
        idx_pad[:n] = idx
        xc = np.ascontiguousarray(x[z][idx_pad].astype(np.float32).T).astype(bf16)
        wa = min(512, N)
        xwp = np.ascontiguousarray(np.concatenate([xc[:, :wa], wm], axis=1))
        kb = np.full(N, -30000.0, dtype=np.float32)
        kb[:n] = 0.0
        # kbias SBUF layout: column j covers compacted rows koff_j..koff_j+kh_j
        kbm = np.full((P, nkt), -30000.0, dtype=np.float32)
        for j, (koff, kh) in enumerate(ktiles):
            kbm[:kh, j] = kb[koff : koff + kh]
        in_maps.append(
            {
                "xwp": xwp,
                "xc": xc,
                "wv": wv_t,
                "bv": bv_row,
                "kbias": np.ascontiguousarray(kbm),
            }
        )
    return in_maps


def run(x, Wq, Wk, Wv, bv, mask, trace=False):
    from concourse.bass_utils import run_bass_kernel_spmd

    x = np.asarray(x)
    mask = np.asarray(mask).astype(bool)
    idxs = [np.nonzero(~mask[z])[0] for z in range(8)]
    nmax = max(int(i.size) for i in idxs)
    N = max(GRAN, -(-nmax // GRAN) * GRAN)  # shared padded length
    nc = _get_nc(N)
    in_maps = _make_in_maps(x, Wq, Wk, Wv, bv, mask, idxs, N)
    res = run_bass_kernel_spmd(nc, in_maps, core_ids=list(range(8)), trace=trace)
    out = np.zeros((8, S, D), dtype=np.float32)
    for z in range(8):
        n = idxs[z].size
        if n:
            out[z][idxs[z]] = res.results[z]["out"][:n].astype(np.float32)
    return out, res


def kernel(x, Wq, Wk, Wv, bv, mask):
    out, _ = run(x, Wq, Wk, Wv, bv, mask, trace=False)
    return out
